# revision 18
# baseline (speedup 1.0000x reference)
"""Trainium2 Bass kernel for nn_DecMoE_19851338842797 (moe_routing).

The reference tiles a single row into x_ds, so the noisy-top-k gating picks the
SAME two experts with the SAME two gates for every token.  The whole MoE then
collapses, per token b, to

    out[b] = log( g_lo * exp(mlp_lo(D[b])) + g_hi * exp(mlp_hi(D[b])) )

where mlp_e(d) = leaky_relu(d @ w1[e] + b1[e], 0.1) @ (w2[e] @ P[e]) + b2[e] @ P[e]
and P[e] scatters the ks*ks outputs into a centered 9x9 grid (a 0/1 matrix, so
folding it into w2/b2 is exact).

Device kernel (per core, data-parallel over tokens, 32768 tokens/core):
  - host pre-transposes the D_Kernel shard to [128 features, 32768 tokens] so the
    tensor engine can contract over features without on-device transposes
  - mm1 (+rank-1 bias matmul) -> hidden [64, tok] stacked 2 token-groups into
    128 partitions; leaky relu via one DVE scalar_tensor_tensor (max(x, 0.1x))
  - mm2 against [w2P_lo] and [w2P_hi - w2P_lo] -> z_lo and d = z_hi - z_lo in
    PSUM, [81 x tokens] (output stays transposed; host transposes back)
  - combine via log-sum-exp: out = (z_lo + beta_lo) + ln(1 + exp(d + dbeta))
    with beta_e = b2P_e + ln(g_e) folded in as per-partition biases
  - gating + load-balance loss are tiny and computed on host.
"""

import sys

import numpy as np

for _p in ("/opt/trn_rl_repo", "/root/.axon_site/_ro/trn_rl_repo"):
    if _p not in sys.path:
        sys.path.append(_p)

B, DS, C, HID, E, K = 262144, 128, 128, 32, 4, 2
KS = [3, 5, 7, 9]
NCORES = 8
BS = B // NCORES        # tokens per core
TB = 1024               # tokens per device loop iteration (2 "supertiles")

_CACHE = {}


# -----------------------------------------------------------------------------
# device program
# -----------------------------------------------------------------------------
def _build_nc():
    import concourse.bacc as bacc
    import concourse.tile as tile
    from concourse import mybir

    f32 = mybir.dt.float32
    AF = mybir.ActivationFunctionType
    OP = mybir.AluOpType

    nc = bacc.Bacc("TRN2")

    dt_ = nc.dram_tensor("dt", [128, BS], f32, kind="ExternalInput")
    # packed constants: [0:64]=w1c, [64:65]=b1cat(x2), [65:227]=w2s,
    # [227:228]=beta_lo (rows 0:81), [228:229]=dbeta (rows 0:81)
    cst = nc.dram_tensor("cst", [128, 229], f32, kind="ExternalInput")
    ot = nc.dram_tensor("ot", [81, BS], f32, kind="ExternalOutput")

    from contextlib import ExitStack

    with tile.TileContext(nc) as tc, ExitStack() as ctx:
        const = ctx.enter_context(tc.tile_pool(name="const", bufs=1))
        dpool = ctx.enter_context(tc.tile_pool(name="dpool", bufs=3))
        hpool = ctx.enter_context(tc.tile_pool(name="hpool", bufs=2))
        epool = ctx.enter_context(tc.tile_pool(name="epool", bufs=2))
        lpool = ctx.enter_context(tc.tile_pool(name="lpool", bufs=2))
        opool = ctx.enter_context(tc.tile_pool(name="opool", bufs=3))
        ps_hp = ctx.enter_context(tc.tile_pool(name="ps_hp", bufs=2, space="PSUM"))
        ps_lp = ctx.enter_context(tc.tile_pool(name="ps_lp", bufs=2, space="PSUM"))
        ps_dp = ctx.enter_context(tc.tile_pool(name="ps_dp", bufs=1, space="PSUM"))

        sb_c = const.tile([128, 229], f32)
        nc.sync.dma_start(out=sb_c, in_=cst[:, :])
        sb_w1c = sb_c[:, 0:64]
        sb_b1r = sb_c[:, 64:65]
        sb_w2s = sb_c[:, 65:227]
        sb_blo = sb_c[0:81, 227:228]
        sb_bde = sb_c[0:81, 228:229]

        for k in range(BS // TB):
            # ---- load [128 features, 1024 tokens] ----
            sb_d = dpool.tile([128, TB], f32)
            nc.sync.dma_start(out=sb_d, in_=dt_[:, k * TB:(k + 1) * TB])
            # token groups: A = halves [0:256] of each 512-block, B = [256:512]
            dv = sb_d.rearrange("p (s h t) -> p s h t", s=2, h=2)

            # ---- hidden: bias (rank-1 matmul) + W1 contraction ----
            # stacked layout: partitions 0:64 = hidden(A tokens), 64:128 = hidden(B)
            ph = ps_hp.tile([128, 512], f32)
            nc.tensor.matmul(ph[0:64, :], sb_w1c, dv[:, :, 0, :])
            nc.tensor.matmul(
                ph[64:128, :], sb_w1c, dv[:, :, 1, :], tile_position=(0, 64),
            )

            # ---- leaky relu with folded b1 bias: h = prelu(x + b1, 0.1) ----
            sb_h = hpool.tile([128, 512], f32)
            nc.scalar.activation(
                out=sb_h, in_=ph, func=AF.Prelu, bias=sb_b1r, alpha=0.1,
            )

            # ---- z_lo and d = z_hi - z_lo, [81, 1024], col order (A0 A1 B0 B1) ----
            pl = ps_lp.tile([81, 1024], f32)
            pd = ps_dp.tile([81, 1024], f32)
            nc.tensor.matmul(pl[:, 0:512], sb_w2s[0:32, 0:81], sb_h[0:32, :])
            nc.tensor.matmul(
                pl[:, 512:1024], sb_w2s[64:96, 0:81], sb_h[64:96, :],
                tile_position=(64, 0),
            )
            nc.tensor.matmul(pd[:, 0:512], sb_w2s[0:64, 81:162], sb_h[0:64, :])
            nc.tensor.matmul(
                pd[:, 512:1024], sb_w2s[64:128, 81:162], sb_h[64:128, :],
                tile_position=(64, 0),
            )

            # ---- combine: out = (z_lo + beta_lo) + ln(1 + exp(d + dbeta)) ----
            sb_e = epool.tile([81, 1024], f32)
            nc.scalar.activation(out=sb_e, in_=pd, func=AF.Exp, bias=sb_bde)
            sb_l = lpool.tile([81, 1024], f32)
            nc.scalar.activation(out=sb_l, in_=sb_e, func=AF.Ln, bias=1.0)
            sb_o = opool.tile([81, 1024], f32)
            nc.vector.scalar_tensor_tensor(
                out=sb_o, in0=pl, scalar=sb_blo, in1=sb_l, op0=OP.add, op1=OP.add,
            )

            # ---- store; undo the (A0 A1 B0 B1) interleave: want (A0 B0 A1 B1) ----
            ov = sb_o.rearrange("p (h s t) -> p s h t", h=2, s=2)
            for s in range(2):
                nc.sync.dma_start(
                    out=ot[:, k * TB + s * 512:k * TB + (s + 1) * 512],
                    in_=ov[:, s, :, :],
                )

    nc.compile()
    return nc


def _get_nc():
    if "nc" not in _CACHE:
        _CACHE["nc"] = _build_nc()
    return _CACHE["nc"]


# -----------------------------------------------------------------------------
# host-side gating / weight folding
# -----------------------------------------------------------------------------
def _pad_w2(w2_e, b2_e, ks):
    """Fold the 0/1 padding matrix P[e] into w2/b2 (exact: P scatters columns)."""
    pad = (9 - ks) // 2
    w2P = np.zeros((HID, 81), np.float32)
    b2P = np.zeros((81,), np.float32)
    for p in range(ks * ks):
        r, c = divmod(p, ks)
        q = (r + pad) * 9 + (c + pad)
        w2P[:, q] = w2_e[:, p]
        b2P[q] = b2_e[p]
    return w2P, b2P


def _gating(x_ds, w_gate):
    logits = (x_ds[0].astype(np.float32) @ w_gate.astype(np.float32)).astype(np.float32)
    order = np.argsort(-logits, kind="stable")
    top2 = order[:K]
    v = logits[top2].astype(np.float32)
    ex = np.exp(v - v.max(), dtype=np.float32)
    g = (ex / ex.sum(dtype=np.float32)).astype(np.float32)
    if top2[0] < top2[1]:
        return int(top2[0]), int(top2[1]), np.float32(g[0]), np.float32(g[1]), top2, g, logits
    return int(top2[1]), int(top2[0]), np.float32(g[1]), np.float32(g[0]), top2, g, logits


def _loss(top2, g):
    gates_row = np.zeros(E, np.float32)
    gates_row[top2] = g
    importance = (np.float32(B) * gates_row).astype(np.float32)
    load = np.where(gates_row > 0, np.float32(B), np.float32(0)).astype(np.float32)

    def cv2(x):
        m = np.mean(x, dtype=np.float32)
        v = np.var(x, ddof=1, dtype=np.float32)
        return np.float32(v / (m * m + np.float32(1e-10)))

    return np.float32((cv2(importance) + cv2(load)) * np.float32(0.01))


def prep_device_inputs(inputs):
    """Host-side gating + weight folding + sharding; returns in_maps and loss."""
    return _prep(
        inputs["x_ds"], inputs["D_Kernel"], inputs["w_gate"],
        inputs["w1"], inputs["b1"], inputs["w2"], inputs["b2"],
    )


def _prep(x_ds, D_Kernel, w_gate, w1, b1, w2, b2):
    e_lo, e_hi, g_lo, g_hi, top2, g, _ = _gating(x_ds, w_gate)
    loss = _loss(top2, g)

    w1c = np.ascontiguousarray(
        np.concatenate([w1[e_lo], w1[e_hi]], axis=1), dtype=np.float32
    )                                                     # [128, 64]
    b1cat = np.concatenate([b1[e_lo], b1[e_hi]]).astype(np.float32)     # [64]
    b1r = np.ascontiguousarray(np.tile(b1cat, 2)[:, None])              # [128, 1]

    w2Pl, b2Pl = _pad_w2(w2[e_lo], b2[e_lo], KS[e_lo])
    w2Ph, b2Ph = _pad_w2(w2[e_hi], b2[e_hi], KS[e_hi])
    w2d = np.concatenate([-w2Pl, w2Ph], axis=0)           # [64, 81]
    w2s = np.zeros((128, 162), np.float32)
    w2s[0:32, 0:81] = w2Pl
    w2s[64:96, 0:81] = w2Pl
    w2s[0:64, 81:162] = w2d
    w2s[64:128, 81:162] = w2d

    beta_l = (b2Pl + np.log(g_lo, dtype=np.float32)).astype(np.float32)
    beta_h = (b2Ph + np.log(g_hi, dtype=np.float32)).astype(np.float32)
    blo = np.ascontiguousarray(beta_l[:, None])           # [81, 1]
    bde = np.ascontiguousarray((beta_h - beta_l)[:, None])  # [81, 1]

    cst = np.zeros((128, 229), np.float32)
    cst[:, 0:64] = w1c
    cst[:, 64:65] = b1r
    cst[:, 65:227] = w2s
    cst[0:81, 227:228] = blo
    cst[0:81, 228:229] = bde

    D = np.asarray(D_Kernel, dtype=np.float32)
    in_maps = []
    for r in range(NCORES):
        in_maps.append({
            "dt": np.ascontiguousarray(D[r * BS:(r + 1) * BS].T),
            "cst": cst,
        })
    return {"in_maps": in_maps, "loss": loss}


def kernel(x_ds, D_Kernel, w_gate, w1, b1, w2, b2):
    from concourse.bass_utils import run_bass_kernel_spmd

    prep = _prep(x_ds, D_Kernel, w_gate, w1, b1, w2, b2)
    nc = _get_nc()
    res = run_bass_kernel_spmd(nc, prep["in_maps"], core_ids=list(range(NCORES)))
    shards = [np.asarray(res.results[r]["ot"]) for r in range(NCORES)]  # [81, BS]
    out = np.concatenate([s.T for s in shards], axis=0)   # [B, 81]
    out = np.ascontiguousarray(out, dtype=np.float32).reshape(B, 1, 9, 9)
    return out, prep["loss"]


# revision 25
# speedup vs baseline: 1.5175x; 1.5175x over previous
"""Trainium2 Bass kernel for nn_DecMoE_19851338842797 (moe_routing).

The reference tiles a single row into x_ds, so the noisy-top-k gating picks the
SAME two experts with the SAME two gates for every token.  The whole MoE then
collapses, per token b, to

    out[b] = log( g_lo * exp(mlp_lo(D[b])) + g_hi * exp(mlp_hi(D[b])) )

where mlp_e(d) = leaky_relu(d @ w1[e] + b1[e], 0.1) @ (w2[e] @ P[e]) + b2[e] @ P[e]
and P[e] scatters the ks*ks outputs into a centered 9x9 grid (a 0/1 matrix, so
folding it into w2/b2 is exact).

Device kernel (per core, data-parallel over tokens, 32768 tokens/core):
  - host pre-transposes the D_Kernel shard to [128 features, 32768 tokens] so the
    tensor engine can contract over features without on-device transposes
  - mm1 (+rank-1 bias matmul) -> hidden [64, tok] stacked 2 token-groups into
    128 partitions; leaky relu via one DVE scalar_tensor_tensor (max(x, 0.1x))
  - mm2 against [w2P_lo] and [w2P_hi - w2P_lo] -> z_lo and d = z_hi - z_lo in
    PSUM, [81 x tokens] (output stays transposed; host transposes back)
  - combine via log-sum-exp: out = (z_lo + beta_lo) + ln(1 + exp(d + dbeta))
    with beta_e = b2P_e + ln(g_e) folded in as per-partition biases
  - gating + load-balance loss are tiny and computed on host.
"""

import sys

import numpy as np

for _p in ("/opt/trn_rl_repo", "/root/.axon_site/_ro/trn_rl_repo"):
    if _p not in sys.path:
        sys.path.append(_p)

B, DS, C, HID, E, K = 262144, 128, 128, 32, 4, 2
KS = [3, 5, 7, 9]
NCORES = 8
BS = B // NCORES        # tokens per core
TB = 1024               # tokens per device loop iteration (2 "supertiles")

_CACHE = {}


# -----------------------------------------------------------------------------
# device program
# -----------------------------------------------------------------------------
def _build_nc():
    import bass_rust as _bass_rust
    import concourse.bacc as bacc
    import concourse.tile as tile
    from concourse import mybir
    from concourse.hw_specs import get_activation_tables

    f32 = mybir.dt.float32
    f32r = mybir.dt.float32r
    AF = mybir.ActivationFunctionType
    OP = mybir.AluOpType

    class OneActSetBacc(bacc.Bacc):
        """Force all activations into natural_log_exp_and_others (covers
        Prelu/Exp/Ln) — the default greedy set choice splits them across two
        sets and pays a 1.3us ACT_TABLE_LOAD twice per loop iteration."""

        def insert_act_table_loads(self):
            has_activation = any(
                isinstance(i, mybir.InstActivation)
                for b in self.main_func.blocks
                for i in b.instructions
            )
            if not has_activation:
                return
            keep = "natural_log_exp_and_others"
            tables = [
                (name, (funcs if name == keep else set()))
                for name, funcs in get_activation_tables(self.m.arch).items()
            ]
            _bass_rust.insert_act_table_loads(self, tables)

    nc = OneActSetBacc("TRN2")

    dt_ = nc.dram_tensor("dt", [128, BS], f32r, kind="ExternalInput")
    # packed constants: [0:64]=w1c, [64:65]=b1cat(x2), [65:227]=w2s,
    # [227:228]=beta_lo (rows 0:81), [228:229]=dbeta (rows 0:81)
    cst = nc.dram_tensor("cst", [128, 229], f32, kind="ExternalInput")
    ot = nc.dram_tensor("ot", [81, BS], f32, kind="ExternalOutput")

    from contextlib import ExitStack

    with tile.TileContext(nc) as tc, ExitStack() as ctx:
        const = ctx.enter_context(tc.tile_pool(name="const", bufs=1))
        dpool = ctx.enter_context(tc.tile_pool(name="dpool", bufs=3))
        hpool = ctx.enter_context(tc.tile_pool(name="hpool", bufs=2))
        epool = ctx.enter_context(tc.tile_pool(name="epool", bufs=2))
        lpool = ctx.enter_context(tc.tile_pool(name="lpool", bufs=2))
        opool = ctx.enter_context(tc.tile_pool(name="opool", bufs=3))
        ps_hp = ctx.enter_context(tc.tile_pool(name="ps_hp", bufs=2, space="PSUM"))
        ps_lp = ctx.enter_context(tc.tile_pool(name="ps_lp", bufs=2, space="PSUM"))
        ps_dp = ctx.enter_context(tc.tile_pool(name="ps_dp", bufs=1, space="PSUM"))

        sb_c = const.tile([128, 229], f32)
        nc.sync.dma_start(out=sb_c, in_=cst[:, :])
        sb_w1c = sb_c[:, 0:64]
        sb_b1r = sb_c[:, 64:65]
        sb_w2s = sb_c[:, 65:227]
        sb_blo = sb_c[0:81, 227:228]
        sb_bde = sb_c[0:81, 228:229]
        # f32r-rounded copies of the matmul weights (one-time)
        sb_w1r = const.tile([128, 64], f32r)
        nc.vector.tensor_copy(out=sb_w1r, in_=sb_w1c)
        sb_w2r = const.tile([128, 162], f32r)
        nc.vector.tensor_copy(out=sb_w2r, in_=sb_w2s)

        for k in range(BS // TB):
            # ---- load [128 features, 1024 tokens] ----
            sb_d = dpool.tile([128, TB], f32r)
            nc.sync.dma_start(out=sb_d, in_=dt_[:, k * TB:(k + 1) * TB])
            # token groups: A = halves [0:256] of each 512-block, B = [256:512]
            dv = sb_d.rearrange("p (s h t) -> p s h t", s=2, h=2)

            # ---- hidden: W1 contraction (f32r: single-pass fp32 matmul) ----
            # stacked layout: partitions 0:64 = hidden(A tokens), 64:128 = hidden(B)
            ph = ps_hp.tile([128, 512], f32)
            nc.tensor.matmul(ph[0:64, :], sb_w1r, dv[:, :, 0, :])
            # f32r + column tile_position fails walrus codegen (NCC_IXCG864);
            # run the second half as plain fp32 (2-pass) instead.
            nc.tensor.matmul(
                ph[64:128, :], sb_w1c, dv[:, :, 1, :].bitcast(f32),
                tile_position=(0, 64),
            )

            # ---- leaky relu with folded b1 bias: h = prelu(x + b1, 0.1) ----
            sb_h = hpool.tile([128, 512], f32r)
            nc.scalar.activation(
                out=sb_h, in_=ph, func=AF.Prelu, bias=sb_b1r, alpha=0.1,
            )

            # ---- z_lo and d = z_hi - z_lo, [81, 1024], col order (A0 A1 B0 B1) ----
            pl = ps_lp.tile([81, 1024], f32)
            pd = ps_dp.tile([81, 1024], f32)
            nc.tensor.matmul(pl[:, 0:512], sb_w2r[0:32, 0:81], sb_h[0:32, :])
            nc.tensor.matmul(
                pl[:, 512:1024], sb_w2r[64:96, 0:81], sb_h[64:96, :],
                tile_position=(64, 0),
            )
            nc.tensor.matmul(pd[:, 0:512], sb_w2r[0:64, 81:162], sb_h[0:64, :])
            nc.tensor.matmul(
                pd[:, 512:1024], sb_w2r[64:128, 81:162], sb_h[64:128, :],
                tile_position=(64, 0),
            )

            # ---- combine: out = (z_lo + beta_lo) + ln(1 + exp(d + dbeta)) ----
            sb_e = epool.tile([81, 1024], f32)
            nc.scalar.activation(out=sb_e, in_=pd, func=AF.Exp, bias=sb_bde)
            sb_l = lpool.tile([81, 1024], f32)
            nc.scalar.activation(out=sb_l, in_=sb_e, func=AF.Ln, bias=1.0)
            sb_o = opool.tile([81, 1024], f32)
            nc.vector.scalar_tensor_tensor(
                out=sb_o, in0=pl, scalar=sb_blo, in1=sb_l, op0=OP.add, op1=OP.add,
            )

            # ---- store; undo the (A0 A1 B0 B1) interleave: want (A0 B0 A1 B1) ----
            ov = sb_o.rearrange("p (h s t) -> p s h t", h=2, s=2)
            for s in range(2):
                nc.sync.dma_start(
                    out=ot[:, k * TB + s * 512:k * TB + (s + 1) * 512],
                    in_=ov[:, s, :, :],
                )

    nc.compile()
    return nc


def _get_nc():
    if "nc" not in _CACHE:
        _CACHE["nc"] = _build_nc()
    return _CACHE["nc"]


# -----------------------------------------------------------------------------
# host-side gating / weight folding
# -----------------------------------------------------------------------------
def _pad_w2(w2_e, b2_e, ks):
    """Fold the 0/1 padding matrix P[e] into w2/b2 (exact: P scatters columns)."""
    pad = (9 - ks) // 2
    w2P = np.zeros((HID, 81), np.float32)
    b2P = np.zeros((81,), np.float32)
    for p in range(ks * ks):
        r, c = divmod(p, ks)
        q = (r + pad) * 9 + (c + pad)
        w2P[:, q] = w2_e[:, p]
        b2P[q] = b2_e[p]
    return w2P, b2P


def _gating(x_ds, w_gate):
    logits = (x_ds[0].astype(np.float32) @ w_gate.astype(np.float32)).astype(np.float32)
    order = np.argsort(-logits, kind="stable")
    top2 = order[:K]
    v = logits[top2].astype(np.float32)
    ex = np.exp(v - v.max(), dtype=np.float32)
    g = (ex / ex.sum(dtype=np.float32)).astype(np.float32)
    if top2[0] < top2[1]:
        return int(top2[0]), int(top2[1]), np.float32(g[0]), np.float32(g[1]), top2, g, logits
    return int(top2[1]), int(top2[0]), np.float32(g[1]), np.float32(g[0]), top2, g, logits


def _loss(top2, g):
    gates_row = np.zeros(E, np.float32)
    gates_row[top2] = g
    importance = (np.float32(B) * gates_row).astype(np.float32)
    load = np.where(gates_row > 0, np.float32(B), np.float32(0)).astype(np.float32)

    def cv2(x):
        m = np.mean(x, dtype=np.float32)
        v = np.var(x, ddof=1, dtype=np.float32)
        return np.float32(v / (m * m + np.float32(1e-10)))

    return np.float32((cv2(importance) + cv2(load)) * np.float32(0.01))


def prep_device_inputs(inputs):
    """Host-side gating + weight folding + sharding; returns in_maps and loss."""
    return _prep(
        inputs["x_ds"], inputs["D_Kernel"], inputs["w_gate"],
        inputs["w1"], inputs["b1"], inputs["w2"], inputs["b2"],
    )


def _prep(x_ds, D_Kernel, w_gate, w1, b1, w2, b2):
    e_lo, e_hi, g_lo, g_hi, top2, g, _ = _gating(x_ds, w_gate)
    loss = _loss(top2, g)

    w1c = np.ascontiguousarray(
        np.concatenate([w1[e_lo], w1[e_hi]], axis=1), dtype=np.float32
    )                                                     # [128, 64]
    b1cat = np.concatenate([b1[e_lo], b1[e_hi]]).astype(np.float32)     # [64]
    b1r = np.ascontiguousarray(np.tile(b1cat, 2)[:, None])              # [128, 1]

    w2Pl, b2Pl = _pad_w2(w2[e_lo], b2[e_lo], KS[e_lo])
    w2Ph, b2Ph = _pad_w2(w2[e_hi], b2[e_hi], KS[e_hi])
    w2d = np.concatenate([-w2Pl, w2Ph], axis=0)           # [64, 81]
    w2s = np.zeros((128, 162), np.float32)
    w2s[0:32, 0:81] = w2Pl
    w2s[64:96, 0:81] = w2Pl
    w2s[0:64, 81:162] = w2d
    w2s[64:128, 81:162] = w2d

    beta_l = (b2Pl + np.log(g_lo, dtype=np.float32)).astype(np.float32)
    beta_h = (b2Ph + np.log(g_hi, dtype=np.float32)).astype(np.float32)
    blo = np.ascontiguousarray(beta_l[:, None])           # [81, 1]
    bde = np.ascontiguousarray((beta_h - beta_l)[:, None])  # [81, 1]

    cst = np.zeros((128, 229), np.float32)
    cst[:, 0:64] = w1c
    cst[:, 64:65] = b1r
    cst[:, 65:227] = w2s
    cst[0:81, 227:228] = blo
    cst[0:81, 228:229] = bde

    D = np.asarray(D_Kernel, dtype=np.float32)
    in_maps = []
    for r in range(NCORES):
        in_maps.append({
            "dt": np.ascontiguousarray(D[r * BS:(r + 1) * BS].T),
            "cst": cst,
        })
    return {"in_maps": in_maps, "loss": loss}


def kernel(x_ds, D_Kernel, w_gate, w1, b1, w2, b2):
    from concourse.bass_utils import run_bass_kernel_spmd

    prep = _prep(x_ds, D_Kernel, w_gate, w1, b1, w2, b2)
    nc = _get_nc()
    res = run_bass_kernel_spmd(nc, prep["in_maps"], core_ids=list(range(NCORES)))
    shards = [np.asarray(res.results[r]["ot"]) for r in range(NCORES)]  # [81, BS]
    out = np.concatenate([s.T for s in shards], axis=0)   # [B, 81]
    out = np.ascontiguousarray(out, dtype=np.float32).reshape(B, 1, 9, 9)
    return out, prep["loss"]
